# revision 21
# baseline (speedup 1.0000x reference)
"""Trainium2 Bass kernel for a 16-head self-attention block.

Model (matches the nn.Module reference):
    q = x @ Wq + bq; k = x @ Wk + bk; v = x @ Wv + bv   (per-head split, Hd=64)
    attn = softmax(q k^T / sqrt(Hd)); out = (attn v) @ Wo + bo
Shapes: x [2, 2048, 1024], 16 heads, head dim 64.

Sharding (8 cores): core = (batch b in {0,1}) x (head-group g in {0..3}),
each core owns 4 heads of one batch element. Inputs are sliced on the host;
each core returns a partial y^T = (attended_g @ Wo_g)^T which the host sums
over the 4 head-groups per batch.

Per-core layout strategy (all fp32):
  - Host passes xT = x[b]^T [1024, 2048] so projections need no on-device
    transpose: Q^T = (Wq_g)^T @ xT via matmul(lhsT=Wq chunk, rhs=xT chunk).
  - Scores are computed transposed, S^T[key, q] = K_h Q_h^T, so softmax's
    exp can be applied straight out of PSUM by the Scalar engine and the
    A = P V matmul consumes P^T with no transpose anywhere.
  - softmax skips the max-subtraction (mathematically identical; scores are
    O(5) here and exp is <=2 ULP on [-10,10]).
  - Row sums of P come from ones-vector matmuls (partition reduction on PE);
    normalization is applied after A·V to [256, 512] tiles instead of to the
    [2048, 512] P tiles (64x less work).
  - 1/sqrt(Hd) is folded into Wq (and bq) on the host; bv and bo are folded
    in exactly on the host: y += bo + bv @ Wo (softmax rows sum to 1).

tile_position packing: the two heads of a pair run concurrently on the PE
(row groups 0-63 / 64-127 for the K=64 score matmuls; col groups for the
M=64 A·V matmuls; four col groups for the M=1 sums matmuls).
"""

import numpy as np

import concourse.bass as bass
import concourse.tile as tile
from concourse import bacc
from concourse import mybir

P = 128          # partitions
S = 2048         # sequence length
D = 1024         # model dim
H = 16           # total heads
HD = 64          # head dim
G = 4            # heads per core
GD = G * HD      # 256 head-group dims per core
NQB = 4          # query blocks
QB = S // NQB    # 512
NKC = S // P     # 16 key chunks
NDC = D // P     # 8 contraction chunks
F32 = mybir.dt.float32

TRACE = False
LAST_RESULTS = None


DEBUG = False


def _build_nc():
    nc = bacc.Bacc(trn_type="TRN2")
    xT = nc.dram_tensor("xT", [D, S], F32, kind="ExternalInput")
    wq = nc.dram_tensor("wq", [D, GD], F32, kind="ExternalInput")
    wk = nc.dram_tensor("wk", [D, GD], F32, kind="ExternalInput")
    wv = nc.dram_tensor("wv", [D, GD], F32, kind="ExternalInput")
    wo = nc.dram_tensor("wo", [GD, D], F32, kind="ExternalInput")
    bias = nc.dram_tensor("bias", [P, 4], F32, kind="ExternalInput")
    yT = nc.dram_tensor("yT", [D, S], F32, kind="ExternalOutput")
    if DEBUG:
        dbg_sums = nc.dram_tensor("dbg_sums", [P, QB], F32, kind="ExternalOutput")
        dbg_bc = nc.dram_tensor("dbg_bc", [2, P, QB], F32, kind="ExternalOutput")
        dbg_rbc = nc.dram_tensor("dbg_rbc", [2, P, QB], F32, kind="ExternalOutput")
        dbg_attn = nc.dram_tensor("dbg_attn", [2, P, QB], F32, kind="ExternalOutput")

    Exp = mybir.ActivationFunctionType.Exp
    Ident = mybir.ActivationFunctionType.Identity

    with tile.TileContext(nc) as tc, \
         tc.tile_pool(name="sb", bufs=1) as sb, \
         tc.tile_pool(name="pt", bufs=3) as ptp, \
         tc.tile_pool(name="small", bufs=2) as smp, \
         tc.tile_pool(name="ps_s", bufs=2, space="PSUM") as ps_s, \
         tc.tile_pool(name="ps_av", bufs=2, space="PSUM") as ps_av, \
         tc.tile_pool(name="ps_sums", bufs=1, space="PSUM") as ps_sums, \
         tc.tile_pool(name="ps_y", bufs=1, space="PSUM") as ps_y, \
         tc.tile_pool(name="tiny", bufs=4) as tnp:

        # ---- persistent SBUF tensors
        wq_sb = sb.tile([P, NDC, GD], F32, tag="wq")
        wk_sb = sb.tile([P, NDC, GD], F32, tag="wk")
        wv_sb = sb.tile([P, NDC, GD], F32, tag="wv")
        wo_sb = sb.tile([P, 2, D], F32, tag="wo")
        bias_sb = sb.tile([P, 4], F32, tag="bias")
        ones_sb = sb.tile([P, 1], F32, tag="ones")
        scratch = sb.tile([P, 1], F32, tag="scratch")
        zlhs_sb = sb.tile([1, P], F32, tag="zlhs")    # zero weights: bank-clear matmul
        zrhs_sb = sb.tile([1, QB], F32, tag="zrhs")
        ones_row = sb.tile([1, HD], F32, tag="ones_row")
        x_sb = [sb.tile([P, S], F32, tag=f"x{d}", name=f"x{d}") for d in range(NDC)]
        kT = [sb.tile([P, S], F32, tag=f"k{p}", name=f"k{p}") for p in range(2)]
        qT = [sb.tile([P, S], F32, tag=f"q{p}", name=f"q{p}") for p in range(2)]
        v_sb = [sb.tile([P, GD], F32, tag=f"v{c}", name=f"v{c}") for c in range(NKC)]

        # ---- input DMAs
        nc.sync.dma_start(out=wk_sb, in_=wk.rearrange("(o p) m -> p o m", p=P))
        nc.sync.dma_start(out=wv_sb, in_=wv.rearrange("(o p) m -> p o m", p=P))
        nc.sync.dma_start(out=wq_sb, in_=wq.rearrange("(o p) m -> p o m", p=P))
        nc.sync.dma_start(out=wo_sb, in_=wo.rearrange("(o p) m -> p o m", p=P))
        nc.sync.dma_start(out=bias_sb, in_=bias[:, :])
        nc.vector.memset(ones_sb, 1.0)
        nc.vector.memset(ones_row, 1.0)
        nc.vector.memset(zlhs_sb, 0.0)
        nc.vector.memset(zrhs_sb, 0.0)

        def zero_bank(ps_ap, start):
            # K=1 all-zero matmul covering the full bank. With start=True it
            # zeroes the bank and opens the accumulation group for all 128
            # partitions at once, so col-packed sub-range matmuls can all use
            # start=False (safe w.r.t. the bank-wide has_written clear on
            # hardware). With start=False it adds 0 and closes the group over
            # the full partition range.
            nc.tensor.matmul(ps_ap, lhsT=zlhs_sb[:], rhs=zrhs_sb[:],
                             start=start, stop=not start)
        # warm the exp table set early so the ~2.7us load overlaps the prologue
        nc.scalar.activation(out=scratch, in_=ones_sb, func=Exp)
        for d in range(NDC):
            nc.sync.dma_start(out=x_sb[d], in_=xT[d * P:(d + 1) * P, :])

        # Pre-observe each weight DMA on the PE with a 1x1 dummy matmul, so
        # real matmuls never need two DMA-queue waits at once (walrus can't
        # encode >1 sync wait on an LDWEIGHTS).
        wtouch_ps = ps_y.tile([1, 4], F32, tag="y", name="wtouch")
        for i, w in enumerate((wk_sb, wv_sb, wq_sb, wo_sb)):
            nc.tensor.matmul(wtouch_ps[:, i:i + 1], lhsT=w[0:1, 0, 0:1],
                             rhs=w[0:1, 0, 0:1], start=True, stop=True)

        # ---- K^T / Q^T projections: dst[p] [128, 2048],
        # rows 64*h2 hold head (2p+h2)'s 64 dims, columns are sequence.
        for w_sb, dst, bcol0 in ((wk_sb, kT, 2), (wq_sb, qT, 0)):
            for p in range(2):
                for nb2 in range(2):           # 1024 wide output slabs
                    ps = ps_s.tile([P, 2, QB], F32, tag="s")
                    for d in range(NDC):
                        for half in range(2):
                            n0 = (2 * nb2 + half) * QB
                            nc.tensor.matmul(
                                ps[:, half],
                                lhsT=w_sb[:, d, p * P:(p + 1) * P],
                                rhs=x_sb[d][:, n0:n0 + QB],
                                start=(d == 0), stop=(d == NDC - 1))
                    # evict with per-partition bias add
                    nc.scalar.activation(
                        out=dst[p][:, nb2 * 1024:(nb2 + 1) * 1024]
                            .rearrange("p (a b) -> p a b", a=2),
                        in_=ps[:],
                        func=Ident,
                        bias=bias_sb[:, bcol0 + p:bcol0 + p + 1],
                        scale=1.0)

        # ---- V projection: v_sb[c] [128 keys, 256], columns = heads*64
        for c in range(NKC):
            ps = ps_av.tile([P, GD], F32, tag="av")
            for d in range(NDC):
                nc.tensor.matmul(
                    ps[:],
                    lhsT=x_sb[d][:, c * P:(c + 1) * P],
                    rhs=wv_sb[:, d, :],
                    start=(d == 0), stop=(d == NDC - 1))
            nc.vector.tensor_copy(out=v_sb[c], in_=ps[:])

        # ---- attention + output projection, one 512-query block at a time
        for qb in range(NQB):
            q0 = qb * QB
            sums_ps = ps_sums.tile([P, QB], F32, tag="sums")
            av_ps = [ps_av.tile([P, QB], F32, tag="av", name="av_ps") for _ in range(2)]
            zero_bank(sums_ps[:], start=True)
            zero_bank(av_ps[0][:], start=True)
            zero_bank(av_ps[1][:], start=True)
            for c in range(NKC):
                c0 = c * P
                for p in range(2):
                    s_ps = ps_s.tile([P, 2, QB], F32, tag="s")
                    for h2 in range(2):
                        base = HD * h2
                        nc.tensor.matmul(
                            s_ps[:, h2],
                            lhsT=kT[p][base:base + HD, c0:c0 + P],
                            rhs=qT[p][base:base + HD, q0:q0 + QB],
                            start=True, stop=True,
                            tile_position=(base, 0))
                    pt = ptp.tile([P, 2, QB], F32, tag="pt")
                    nc.scalar.activation(out=pt[:], in_=s_ps[:], func=Exp)
                    for h2 in range(2):
                        h = 2 * p + h2
                        nc.tensor.matmul(
                            av_ps[p][HD * h2:HD * (h2 + 1), :],
                            lhsT=v_sb[c][:, h * HD:(h + 1) * HD],
                            rhs=pt[:, h2],
                            start=False, stop=False,
                            tile_position=(0, HD * h2))
                        nc.tensor.matmul(
                            sums_ps[32 * h:32 * h + 1, :],
                            lhsT=ones_sb[:],
                            rhs=pt[:, h2],
                            start=False, stop=False,
                            tile_position=(0, 32 * h))

            zero_bank(sums_ps[:], start=False)
            zero_bank(av_ps[0][:], start=False)
            zero_bank(av_ps[1][:], start=False)

            # normalization: 1/sums, broadcast to each head's 64 rows.
            # Move each sums row (partitions 0/32/64/96) to partition 0 with a
            # tiny SBUF->SBUF DMA, take the reciprocal there, then broadcast it
            # with a K=1 ones outer-product on the PE (two col groups / bank).
            sums_sb = smp.tile([P, QB], F32, tag="sums_sb")
            nc.vector.tensor_copy(out=sums_sb, in_=sums_ps[:])
            rcp = []
            for h in range(G):
                rr = tnp.tile([1, QB], F32, tag="rr", name=f"rr{h}")
                nc.sync.dma_start(out=rr[:, :], in_=sums_sb[32 * h:32 * h + 1, :])
                rc = tnp.tile([1, QB], F32, tag="rcp", name=f"rcp{h}")
                nc.vector.reciprocal(out=rc, in_=rr)
                rcp.append(rc)
            attn = []
            for p in range(2):
                bc_ps = ps_y.tile([P, QB], F32, tag="y", name="bc_ps")
                zero_bank(bc_ps[:], start=True)
                for h2 in range(2):
                    nc.tensor.matmul(
                        bc_ps[HD * h2:HD * (h2 + 1), :],
                        lhsT=ones_row[:],
                        rhs=rcp[2 * p + h2][:],
                        start=False, stop=False,
                        tile_position=(0, HD * h2))
                zero_bank(bc_ps[:], start=False)
                bc_sb = smp.tile([P, QB], F32, tag="bc")
                nc.vector.tensor_copy(out=bc_sb, in_=bc_ps[:])
                at = smp.tile([P, QB], F32, tag="attn")
                nc.vector.tensor_tensor(out=at, in0=av_ps[p][:], in1=bc_sb,
                                        op=mybir.AluOpType.mult)
                attn.append(at)
                if DEBUG and qb == 0:
                    nc.sync.dma_start(out=dbg_bc[p], in_=bc_sb[:])
                    nc.sync.dma_start(out=dbg_attn[p], in_=at[:])
            if DEBUG and qb == 0:
                nc.sync.dma_start(out=dbg_sums[:, :], in_=sums_sb[:])

            # output projection: y^T[m-chunk, qb] = sum_p Wo_p^T @ attn_p
            for m in range(NDC):
                yp = ps_y.tile([P, QB], F32, tag="y")
                for p in range(2):
                    nc.tensor.matmul(
                        yp[:],
                        lhsT=wo_sb[:, p, m * P:(m + 1) * P],
                        rhs=attn[p][:],
                        start=(p == 0), stop=(p == 1))
                ysb = smp.tile([P, QB], F32, tag="ysb")
                nc.vector.tensor_copy(out=ysb, in_=yp[:])
                nc.sync.dma_start(out=yT[m * P:(m + 1) * P, q0:q0 + QB], in_=ysb)

    nc.compile()
    return nc


_CACHE = {}


def _get_nc():
    if "nc" not in _CACHE:
        _CACHE["nc"] = _build_nc()
    return _CACHE["nc"]


def make_in_maps(x, Wq, bq, Wk, bk, Wv, bv, Wo, bo):
    """Host-side sharding: per-core input dicts for cores 0..7."""
    x = np.asarray(x, np.float32)
    scale = np.float32(1.0 / np.sqrt(HD))
    Wq_s = np.asarray(Wq, np.float32) * scale
    bq_s = np.asarray(bq, np.float32) * scale
    Wk = np.asarray(Wk, np.float32)
    bk = np.asarray(bk, np.float32)
    Wv = np.asarray(Wv, np.float32)
    Wo = np.asarray(Wo, np.float32)

    xts = [np.ascontiguousarray(x[b].T) for b in range(2)]
    in_maps = []
    for core in range(8):
        b, g = divmod(core, 4)
        cols = slice(g * GD, (g + 1) * GD)
        bias = np.zeros((P, 4), np.float32)
        bias[:, 0] = bq_s[g * GD:g * GD + P]
        bias[:, 1] = bq_s[g * GD + P:(g + 1) * GD]
        bias[:, 2] = bk[g * GD:g * GD + P]
        bias[:, 3] = bk[g * GD + P:(g + 1) * GD]
        in_maps.append({
            "xT": xts[b],
            "wq": np.ascontiguousarray(Wq_s[:, cols]),
            "wk": np.ascontiguousarray(Wk[:, cols]),
            "wv": np.ascontiguousarray(Wv[:, cols]),
            "wo": np.ascontiguousarray(Wo[cols, :]),
            "bias": bias,
        })
    return in_maps


def gather_output(results, Wv, bv, Wo, bo):
    """Sum per-core partial y^T outputs and fold bv/bo exactly."""
    y = np.zeros((2, S, D), np.float32)
    for core in range(8):
        b = core // 4
        y[b] += results[core]["yT"].T
    y += np.asarray(bo, np.float32) + np.asarray(bv, np.float32) @ np.asarray(Wo, np.float32)
    return y


def kernel(x, Wq, bq, Wk, bk, Wv, bv, Wo, bo):
    global LAST_RESULTS
    from concourse.bass_utils import run_bass_kernel_spmd
    in_maps = make_in_maps(x, Wq, bq, Wk, bk, Wv, bv, Wo, bo)
    res = run_bass_kernel_spmd(_get_nc(), in_maps, core_ids=list(range(8)),
                               trace=TRACE)
    LAST_RESULTS = res
    return gather_output(res.results, Wv, bv, Wo, bo)
